# revision 15
# baseline (speedup 1.0000x reference)
"""Trainium2 Bass kernel for nn_CrossAttention (B=4, Q=1024, T=4096, D=1024, H=16).

Sharding: core = b*2 + g  (b in 0..3 batches, g in 0..1 head-groups of 8 heads).
Each core computes, for its (batch, head-group):
  qT = (Wq_g @ x_q.T)          [512, Q]   (feature-major; head pairs stacked)
  kT = (Wk_g @ x_kv.T)         [512, T]
  v  = (x_kv @ Wv_g.T)         [T, 512]
  sT = k_h @ q_h.T             [T, Q] per head  (scores transposed)
  p  = exp(sT / 8)             (softmax w/o max-subtraction; scores ~N(0,1))
  outT_h = v_h.T @ p ; sums_h = ones.T @ p ; attnT_h = outT_h * (1/sums_h)
  yT_partial = Wo[:, gblock].T.T @ attnT  -> [1024, Q]  fp32
Host sums the two head-group partials per batch and transposes.

The emission is software-pipelined: the attention loop over kv tiles is
ScalarE(exp)-bound, so all projection work (k-proj of the next head pair,
v/q projections, o-proj) is sliced into small actions and emitted inside
the attention loop as TensorE filler, paced so producers stay ahead of
their consumers.
"""

import sys

import numpy as np

for _p in ("/opt/trn_rl_repo",):
    if _p not in sys.path:
        sys.path.insert(0, _p)

import ml_dtypes

import concourse.bass as bass
import concourse.tile as tile
from concourse import bacc, mybir
from concourse.bass_utils import run_bass_kernel_spmd

BF16 = mybir.dt.bfloat16
F32 = mybir.dt.float32
NPBF16 = np.dtype(ml_dtypes.bfloat16)

D = 1024          # model dim
Q = 1024          # query length
T = 4096          # kv length
B = 4             # batch
H = 16            # heads
DH = 64           # head dim
NCORES = 8
G = 2             # head groups (cores per batch)
F = D // G        # features per core = 512
P = 128
ND = D // P       # 8 d-tiles (contraction tiles for projections)
NM = F // P       # 4 feature tiles (head pairs)
NQC = Q // 512    # 2 query chunks
NTC = T // 512    # 8 kv chunks
NTT = T // P      # 32 kv tiles
SCALE = DH ** -0.5


def _emit_kernel(nc, tc, xqT, xkT, wqT, wkT, wvT, woT, yT):
    from contextlib import ExitStack

    ctx = ExitStack()
    with ctx:
        wp = ctx.enter_context(tc.tile_pool(name="wp", bufs=1))
        xp = ctx.enter_context(tc.tile_pool(name="xp", bufs=2))
        st = ctx.enter_context(tc.tile_pool(name="st", bufs=1))
        exp_pool = ctx.enter_context(tc.tile_pool(name="exp", bufs=3))
        small = ctx.enter_context(tc.tile_pool(name="small", bufs=2))
        yop = ctx.enter_context(tc.tile_pool(name="yop", bufs=4))
        psp = ctx.enter_context(tc.tile_pool(name="psp", bufs=1, space="PSUM"))

        # ---- resident weights / activations ----
        wq_sb = wp.tile([P, ND, F], BF16, name="wq_sb", tag="wq")
        wk_sb = wp.tile([P, ND, F], BF16, name="wk_sb", tag="wk")
        wv_sb = wp.tile([P, ND, F], BF16, name="wv_sb", tag="wv")
        wo_sb = wp.tile([P, NM, D], BF16, name="wo_sb", tag="wo")
        qT_sb = st.tile([P, NM, Q], BF16, name="qT_sb", tag="qT")
        kT_sb = st.tile([P, NM, T], BF16, name="kT_sb", tag="kT")
        v_sb = st.tile([P, NTT, F], BF16, name="v_sb", tag="v")
        at_sb = st.tile([P, NM, Q], BF16, name="at_sb", tag="at")
        ones64 = st.tile([P, DH], BF16, name="ones64", tag="ones")

        def wdma(w_sb, wT, n):
            def act():
                for d in range(n):
                    nc.sync.dma_start(
                        out=w_sb[:, d, :], in_=wT[d * P:(d + 1) * P, :]
                    )
            return act

        # ---- projection emitters: (pre_action, [compute actions]) ----
        def kproj_chunk(p, tc_i):
            state = {}

            def dma():
                xk2 = xp.tile([P, ND, 512], BF16, name="xk2", tag="xk2")
                for d in range(ND):
                    nc.sync.dma_start(
                        out=xk2[:, d, :],
                        in_=xkT[d * P:(d + 1) * P,
                                tc_i * 512:(tc_i + 1) * 512],
                    )
                state["xk2"] = xk2

            comp = []

            def alloc():
                state["pk"] = psp.tile([P, 512], F32, name="pk", tag="pp",
                                       bufs=2)

            comp.append(alloc)
            for d in range(ND):
                def mm(d=d):
                    nc.tensor.matmul(
                        state["pk"],
                        lhsT=wk_sb[:, d, p * P:(p + 1) * P],
                        rhs=state["xk2"][:, d, :],
                        start=(d == 0),
                        stop=(d == ND - 1),
                    )
                comp.append(mm)

            def cp():
                nc.vector.tensor_copy(
                    out=kT_sb[:, p, tc_i * 512:(tc_i + 1) * 512],
                    in_=state["pk"],
                )
            comp.append(cp)
            return dma, comp

        def vproj_chunk(tc_i):
            state = {}

            def dma():
                xk_t = xp.tile([P, ND, 512], BF16, name="xk_t", tag="xk")
                for d in range(ND):
                    nc.sync.dma_start(
                        out=xk_t[:, d, :],
                        in_=xkT[d * P:(d + 1) * P,
                                tc_i * 512:(tc_i + 1) * 512],
                    )
                state["xk"] = xk_t

            comp = []
            for j in range(4):
                def alloc(j=j):
                    state[j] = psp.tile([P, 512], F32, name="pv", tag="pp",
                                        bufs=2)
                comp.append(alloc)
                for d in range(ND):
                    def mm(j=j, d=d):
                        nc.tensor.matmul(
                            state[j],
                            lhsT=state["xk"][:, d, j * P:(j + 1) * P],
                            rhs=wv_sb[:, d, :],
                            start=(d == 0),
                            stop=(d == ND - 1),
                        )
                    comp.append(mm)

                def cp(j=j):
                    nc.vector.tensor_copy(
                        out=v_sb[:, tc_i * 4 + j, :], in_=state[j]
                    )
                comp.append(cp)
            return dma, comp

        def qproj(qc):
            state = {}

            def dma():
                xq_t = xp.tile([P, ND, 512], BF16, name="xq_t", tag="xq")
                for d in range(ND):
                    nc.sync.dma_start(
                        out=xq_t[:, d, :],
                        in_=xqT[d * P:(d + 1) * P, qc * 512:(qc + 1) * 512],
                    )
                state["xq"] = xq_t

            comp = []
            for m in range(NM):
                def alloc(m=m):
                    state[m] = psp.tile([P, 512], F32, name="pq", tag="pp",
                                        bufs=2)
                comp.append(alloc)
                for d in range(ND):
                    def mm(m=m, d=d):
                        nc.tensor.matmul(
                            state[m],
                            lhsT=wq_sb[:, d, m * P:(m + 1) * P],
                            rhs=state["xq"][:, d, :],
                            start=(d == 0),
                            stop=(d == ND - 1),
                        )
                    comp.append(mm)

                def cp(m=m):
                    nc.vector.tensor_copy(
                        out=qT_sb[:, m, qc * 512:(qc + 1) * 512],
                        in_=state[m],
                    )
                comp.append(cp)
            return dma, comp

        def oproj_group(m8, qc):
            state = {}
            comp = []

            def alloc():
                state["py"] = psp.tile([P, 512], F32, name="py", tag="pp",
                                       bufs=2)
            comp.append(alloc)
            for k in range(NM):
                def mm(k=k):
                    nc.tensor.matmul(
                        state["py"],
                        lhsT=wo_sb[:, k, m8 * P:(m8 + 1) * P],
                        rhs=at_sb[:, k, qc * 512:(qc + 1) * 512],
                        start=(k == 0),
                        stop=(k == NM - 1),
                    )
                comp.append(mm)

            def st_dma():
                y_t = yop.tile([P, 512], F32, name="y_t", tag="y")
                nc.vector.tensor_copy(out=y_t, in_=state["py"])
                nc.sync.dma_start(
                    out=yT[m8 * P:(m8 + 1) * P, qc * 512:(qc + 1) * 512],
                    in_=y_t,
                )
            comp.append(st_dma)
            return None, comp

        def run(pre, comp):
            if pre is not None:
                pre()
            for a in comp:
                a()

        def spread(pairs, nsteps, lead=4):
            """Evenly distribute (pre, comp) groups over nsteps slots;
            pre (DMA) actions are placed `lead` slots before the group's
            first compute action."""
            sched = [[] for _ in range(nsteps)]
            total = sum(len(c) for _, c in pairs) or 1
            pos = 0
            for pre, comp in pairs:
                first = (pos * nsteps) // total
                if pre is not None:
                    sched[max(0, first - lead)].append(pre)
                for a in comp:
                    sched[min(nsteps - 1, (pos * nsteps) // total)].append(a)
                    pos += 1
            return sched

        # ================= prologue =================
        # DMA order matches first consumers: xq+wq (q-proj), xk+wv
        # (v-proj chunk 0), then wk (k-proj pair 0).
        nc.vector.memset(ones64, 1.0)
        qp0 = qproj(0)
        qp0[0]()                      # xq chunk 0 DMA first
        wdma(wq_sb, wqT, ND)()
        vchunks = [vproj_chunk(c) for c in range(NTC)]
        vchunks[0][0]()               # xk chunk 0 DMA
        wdma(wv_sb, wvT, ND)()
        wdma(wk_sb, wkT, ND)()
        for a in qp0[1]:
            a()
        for a in vchunks[0][1]:
            a()
        kp0 = [kproj_chunk(0, c) for c in range(NTC)]
        run(*kp0[0])

        # deadline-driven schedule for pair-0/qc0: chunk c of k-proj(p0)
        # and v-proj must be emitted by step 4c (their consumers); DMAs
        # go 8 steps early, compute spread over the 4 preceding steps.
        p0sched = [[] for _ in range(NTT)]
        for c in range(1, NTC):
            for pre, comp in (kp0[c], vchunks[c]):
                p0sched[max(0, 4 * c - 8)].append(pre)
                base = 4 * (c - 1)
                n = len(comp)
                for si in range(4):
                    lo, hi = (n * si) // 4, (n * (si + 1)) // 4
                    p0sched[base + si].extend(comp[lo:hi])
        qp1 = qproj(1)
        p0sched[0].append(qp1[0])
        for i, a in enumerate(qp1[1]):
            p0sched[1 + (i * 27) // len(qp1[1])].append(a)
        p0sched[8].append(wdma(wo_sb, woT, NM))

        # per-(pair, qc) filler schedules
        sched = {(0, 0): p0sched}
        sched[(0, 1)] = spread([kproj_chunk(1, c) for c in range(NTC)], NTT)
        s64 = spread([kproj_chunk(2, c) for c in range(NTC)], 2 * NTT)
        sched[(1, 0)], sched[(1, 1)] = s64[:NTT], s64[NTT:]
        s64 = spread([kproj_chunk(3, c) for c in range(NTC)], 2 * NTT)
        sched[(2, 0)], sched[(2, 1)] = s64[:NTT], s64[NTT:]
        sched[(3, 0)] = [[] for _ in range(NTT)]
        sched[(3, 1)] = spread([oproj_group(m8, 0) for m8 in range(D // P)],
                               NTT)

        # ================= attention (software-pipelined) ========
        for p in range(NM):
            for qc in range(NQC):
                qs = slice(qc * 512, (qc + 1) * 512)
                pvt = psp.tile([P, 512], F32, name="pvt", tag="pv", bufs=1)
                smt = psp.tile([P, 512], F32, name="smt", tag="sum", bufs=1)
                prev_ex = None

                def pv_sums(ex, t, p=p, pvt=pvt, smt=smt):
                    nc.tensor.matmul(
                        pvt[0:DH, :],
                        lhsT=v_sb[:, t, p * P:p * P + DH],
                        rhs=ex[:, 0, :],
                        start=(t == 0),
                        stop=(t == NTT - 1),
                        tile_position=(0, 0),
                    )
                    nc.tensor.matmul(
                        pvt[DH:P, :],
                        lhsT=v_sb[:, t, p * P + DH:(p + 1) * P],
                        rhs=ex[:, 1, :],
                        start=(t == 0),
                        stop=(t == NTT - 1),
                        tile_position=(0, 64),
                        skip_group_check=True,
                    )
                    nc.tensor.matmul(
                        smt[0:DH, :],
                        lhsT=ones64,
                        rhs=ex[:, 0, :],
                        start=(t == 0),
                        stop=(t == NTT - 1),
                        tile_position=(0, 0),
                    )
                    nc.tensor.matmul(
                        smt[DH:P, :],
                        lhsT=ones64,
                        rhs=ex[:, 1, :],
                        start=(t == 0),
                        stop=(t == NTT - 1),
                        tile_position=(0, 64),
                        skip_group_check=True,
                    )

                loop_sched = sched[(p, qc)]
                for t in range(NTT):
                    ts = slice(t * P, (t + 1) * P)
                    s_ps = psp.tile([P, 2, 512], F32, name="s_ps", tag="ps",
                                    bufs=2)
                    for hb in range(2):
                        base = 64 * hb
                        nc.tensor.matmul(
                            s_ps[:, hb, :],
                            lhsT=kT_sb[base:base + DH, p, ts],
                            rhs=qT_sb[base:base + DH, p, qs],
                            start=True,
                            stop=True,
                            tile_position=(base, 0),
                        )
                    ex = exp_pool.tile([P, 2, 512], BF16, name="ex", tag="ex")
                    nc.scalar.activation(
                        out=ex,
                        in_=s_ps,
                        func=mybir.ActivationFunctionType.Exp,
                        scale=SCALE,
                    )
                    # software-pipelined PV+sums for the previous t
                    if prev_ex is not None:
                        pv_sums(prev_ex, t - 1)
                    # PE filler last: gives exp(t) headroom before QK(t+1)
                    for a in loop_sched[t]:
                        a()
                    prev_ex = ex
                pv_sums(prev_ex, NTT - 1)
                # normalize: attnT = outT * (1/sums)
                rec = small.tile([P, 512], F32, name="rec", tag="rec")
                nc.vector.reciprocal_approx_fast(out=rec, in_=smt)
                nc.vector.tensor_mul(at_sb[:, p, qs], pvt[:, :], rec)

        # ================= coda: o-projection for qc1 =================
        for m8 in range(D // P):
            run(*oproj_group(m8, 1))


_CACHED_NC = None


def build_program():
    global _CACHED_NC
    if _CACHED_NC is not None:
        return _CACHED_NC
    nc = bacc.Bacc(
        "TRN2", target_bir_lowering=False, debug=False, num_devices=NCORES
    )
    xqT = nc.dram_tensor("xqT", [D, Q], BF16, kind="ExternalInput").ap()
    xkT = nc.dram_tensor("xkT", [D, T], BF16, kind="ExternalInput").ap()
    wqT = nc.dram_tensor("wqT", [D, F], BF16, kind="ExternalInput").ap()
    wkT = nc.dram_tensor("wkT", [D, F], BF16, kind="ExternalInput").ap()
    wvT = nc.dram_tensor("wvT", [D, F], BF16, kind="ExternalInput").ap()
    woT = nc.dram_tensor("woT", [F, D], BF16, kind="ExternalInput").ap()
    yT = nc.dram_tensor("yT", [D, Q], F32, kind="ExternalOutput").ap()
    with tile.TileContext(nc) as tc:
        _emit_kernel(nc, tc, xqT, xkT, wqT, wkT, wvT, woT, yT)
    nc.compile()
    _CACHED_NC = nc
    return nc


def make_in_maps(q_in, kv_in, Wq, Wk, Wv, Wo):
    """Shard + transpose + cast on host. Core = b*2 + g."""
    in_maps = []
    xqTs, xkTs = [], []
    for b in range(B):
        xqTs.append(np.ascontiguousarray(q_in[b].T).astype(NPBF16))
        xkTs.append(np.ascontiguousarray(kv_in[b].T).astype(NPBF16))
    w_parts = []
    for g in range(G):
        blk = slice(g * F, (g + 1) * F)
        w_parts.append(
            dict(
                wqT=np.ascontiguousarray(Wq[blk, :].T).astype(NPBF16),
                wkT=np.ascontiguousarray(Wk[blk, :].T).astype(NPBF16),
                wvT=np.ascontiguousarray(Wv[blk, :].T).astype(NPBF16),
                woT=np.ascontiguousarray(Wo[:, blk].T).astype(NPBF16),
            )
        )
    for b in range(B):
        for g in range(G):
            m = dict(xqT=xqTs[b], xkT=xkTs[b])
            m.update(w_parts[g])
            in_maps.append(m)
    return in_maps


def assemble_output(results):
    """results: list of per-core dicts with 'yT' [D, Q] fp32 partials."""
    out = np.empty((B, Q, D), dtype=np.float32)
    for b in range(B):
        acc = results[2 * b]["yT"] + results[2 * b + 1]["yT"]
        out[b] = acc.T
    return out


def kernel(q_in, kv_in, Wq, Wk, Wv, Wo):
    q_in = np.asarray(q_in, dtype=np.float32)
    kv_in = np.asarray(kv_in, dtype=np.float32)
    Wq = np.asarray(Wq, dtype=np.float32)
    Wk = np.asarray(Wk, dtype=np.float32)
    Wv = np.asarray(Wv, dtype=np.float32)
    Wo = np.asarray(Wo, dtype=np.float32)
    nc = build_program()
    in_maps = make_in_maps(q_in, kv_in, Wq, Wk, Wv, Wo)
    res = run_bass_kernel_spmd(nc, in_maps, list(range(NCORES)))
    return assemble_output(res.results)


# revision 16
# speedup vs baseline: 1.0309x; 1.0309x over previous
"""Trainium2 Bass kernel for nn_CrossAttention (B=4, Q=1024, T=4096, D=1024, H=16).

Sharding: core = b*2 + g  (b in 0..3 batches, g in 0..1 head-groups of 8 heads).
Each core computes, for its (batch, head-group):
  qT = (Wq_g @ x_q.T)          [512, Q]   (feature-major; head pairs stacked)
  kT = (Wk_g @ x_kv.T)         [512, T]
  v  = (x_kv @ Wv_g.T)         [T, 512]
  sT = k_h @ q_h.T             [T, Q] per head  (scores transposed)
  p  = exp(sT / 8)             (softmax w/o max-subtraction; scores ~N(0,1))
  outT_h = v_h.T @ p ; sums_h = ones.T @ p ; attnT_h = outT_h * (1/sums_h)
  yT_partial = Wo[:, gblock].T.T @ attnT  -> [1024, Q]  fp32
Host sums the two head-group partials per batch and transposes.

The emission is software-pipelined: the attention loop over kv tiles is
ScalarE(exp)-bound, so all projection work (k-proj of the next head pair,
v/q projections, o-proj) is sliced into small actions and emitted inside
the attention loop as TensorE filler, paced so producers stay ahead of
their consumers.
"""

import sys

import numpy as np

for _p in ("/opt/trn_rl_repo",):
    if _p not in sys.path:
        sys.path.insert(0, _p)

import ml_dtypes

import concourse.bass as bass
import concourse.tile as tile
from concourse import bacc, mybir
from concourse.bass_utils import run_bass_kernel_spmd

BF16 = mybir.dt.bfloat16
F32 = mybir.dt.float32
NPBF16 = np.dtype(ml_dtypes.bfloat16)

D = 1024          # model dim
Q = 1024          # query length
T = 4096          # kv length
B = 4             # batch
H = 16            # heads
DH = 64           # head dim
NCORES = 8
G = 2             # head groups (cores per batch)
F = D // G        # features per core = 512
P = 128
ND = D // P       # 8 d-tiles (contraction tiles for projections)
NM = F // P       # 4 feature tiles (head pairs)
NQC = Q // 512    # 2 query chunks
NTC = T // 512    # 8 kv chunks
NTT = T // P      # 32 kv tiles
SCALE = DH ** -0.5


def _emit_kernel(nc, tc, xqT, xkT, wqT, wkT, wvT, woT, yT):
    from contextlib import ExitStack

    ctx = ExitStack()
    with ctx:
        wp = ctx.enter_context(tc.tile_pool(name="wp", bufs=1))
        xp = ctx.enter_context(tc.tile_pool(name="xp", bufs=2))
        st = ctx.enter_context(tc.tile_pool(name="st", bufs=1))
        exp_pool = ctx.enter_context(tc.tile_pool(name="exp", bufs=3))
        small = ctx.enter_context(tc.tile_pool(name="small", bufs=2))
        yop = ctx.enter_context(tc.tile_pool(name="yop", bufs=4))
        psp = ctx.enter_context(tc.tile_pool(name="psp", bufs=1, space="PSUM"))

        # ---- resident weights / activations ----
        wq_sb = wp.tile([P, ND, F], BF16, name="wq_sb", tag="wq")
        wk_sb = wp.tile([P, ND, F], BF16, name="wk_sb", tag="wk")
        wv_sb = wp.tile([P, ND, F], BF16, name="wv_sb", tag="wv")
        wo_sb = wp.tile([P, NM, D], BF16, name="wo_sb", tag="wo")
        qT_sb = st.tile([P, NM, Q], BF16, name="qT_sb", tag="qT")
        kT_sb = st.tile([P, NM, T], BF16, name="kT_sb", tag="kT")
        v_sb = st.tile([P, NTT, F], BF16, name="v_sb", tag="v")
        at_sb = st.tile([P, NM, Q], BF16, name="at_sb", tag="at")
        ones64 = st.tile([P, DH], BF16, name="ones64", tag="ones")

        def wdma(w_sb, wT, n):
            def act():
                for d in range(n):
                    nc.sync.dma_start(
                        out=w_sb[:, d, :], in_=wT[d * P:(d + 1) * P, :]
                    )
            return act

        # ---- projection emitters: (pre_action, [compute actions]) ----
        def kproj_chunk(p, tc_i):
            state = {}

            def dma():
                xk2 = xp.tile([P, ND, 512], BF16, name="xk2", tag="xk2")
                for d in range(ND):
                    nc.sync.dma_start(
                        out=xk2[:, d, :],
                        in_=xkT[d * P:(d + 1) * P,
                                tc_i * 512:(tc_i + 1) * 512],
                    )
                state["xk2"] = xk2

            comp = []

            def alloc():
                state["pk"] = psp.tile([P, 512], F32, name="pk", tag="pp",
                                       bufs=2)

            comp.append(alloc)
            for d in range(ND):
                def mm(d=d):
                    nc.tensor.matmul(
                        state["pk"],
                        lhsT=wk_sb[:, d, p * P:(p + 1) * P],
                        rhs=state["xk2"][:, d, :],
                        start=(d == 0),
                        stop=(d == ND - 1),
                    )
                comp.append(mm)

            def cp():
                nc.vector.tensor_copy(
                    out=kT_sb[:, p, tc_i * 512:(tc_i + 1) * 512],
                    in_=state["pk"],
                )
            comp.append(cp)
            return dma, comp

        def vproj_chunk(tc_i):
            state = {}

            def dma():
                xk_t = xp.tile([P, ND, 512], BF16, name="xk_t", tag="xk")
                for d in range(ND):
                    nc.sync.dma_start(
                        out=xk_t[:, d, :],
                        in_=xkT[d * P:(d + 1) * P,
                                tc_i * 512:(tc_i + 1) * 512],
                    )
                state["xk"] = xk_t

            comp = []
            for j in range(4):
                def alloc(j=j):
                    state[j] = psp.tile([P, 512], F32, name="pv", tag="pp",
                                        bufs=2)
                comp.append(alloc)
                for d in range(ND):
                    def mm(j=j, d=d):
                        nc.tensor.matmul(
                            state[j],
                            lhsT=state["xk"][:, d, j * P:(j + 1) * P],
                            rhs=wv_sb[:, d, :],
                            start=(d == 0),
                            stop=(d == ND - 1),
                        )
                    comp.append(mm)

                def cp(j=j):
                    nc.vector.tensor_copy(
                        out=v_sb[:, tc_i * 4 + j, :], in_=state[j]
                    )
                comp.append(cp)
            return dma, comp

        def qproj(qc):
            state = {}

            def dma():
                xq_t = xp.tile([P, ND, 512], BF16, name="xq_t", tag="xq")
                for d in range(ND):
                    nc.sync.dma_start(
                        out=xq_t[:, d, :],
                        in_=xqT[d * P:(d + 1) * P, qc * 512:(qc + 1) * 512],
                    )
                state["xq"] = xq_t

            comp = []
            for m in range(NM):
                def alloc(m=m):
                    state[m] = psp.tile([P, 512], F32, name="pq", tag="pp",
                                        bufs=2)
                comp.append(alloc)
                for d in range(ND):
                    def mm(m=m, d=d):
                        nc.tensor.matmul(
                            state[m],
                            lhsT=wq_sb[:, d, m * P:(m + 1) * P],
                            rhs=state["xq"][:, d, :],
                            start=(d == 0),
                            stop=(d == ND - 1),
                        )
                    comp.append(mm)

                def cp(m=m):
                    nc.vector.tensor_copy(
                        out=qT_sb[:, m, qc * 512:(qc + 1) * 512],
                        in_=state[m],
                    )
                comp.append(cp)
            return dma, comp

        def oproj_group(m8, qc):
            state = {}
            comp = []

            def alloc():
                state["py"] = psp.tile([P, 512], F32, name="py", tag="pp",
                                       bufs=2)
            comp.append(alloc)
            for k in range(NM):
                def mm(k=k):
                    nc.tensor.matmul(
                        state["py"],
                        lhsT=wo_sb[:, k, m8 * P:(m8 + 1) * P],
                        rhs=at_sb[:, k, qc * 512:(qc + 1) * 512],
                        start=(k == 0),
                        stop=(k == NM - 1),
                    )
                comp.append(mm)

            def st_dma():
                y_t = yop.tile([P, 512], F32, name="y_t", tag="y")
                nc.vector.tensor_copy(out=y_t, in_=state["py"])
                nc.sync.dma_start(
                    out=yT[m8 * P:(m8 + 1) * P, qc * 512:(qc + 1) * 512],
                    in_=y_t,
                )
            comp.append(st_dma)
            return None, comp

        def run(pre, comp):
            if pre is not None:
                pre()
            for a in comp:
                a()

        def spread(pairs, nsteps, lead=4):
            """Evenly distribute (pre, comp) groups over nsteps slots;
            pre (DMA) actions are placed `lead` slots before the group's
            first compute action."""
            sched = [[] for _ in range(nsteps)]
            total = sum(len(c) for _, c in pairs) or 1
            pos = 0
            for pre, comp in pairs:
                first = (pos * nsteps) // total
                if pre is not None:
                    sched[max(0, first - lead)].append(pre)
                for a in comp:
                    sched[min(nsteps - 1, (pos * nsteps) // total)].append(a)
                    pos += 1
            return sched

        # ================= prologue =================
        # DMA order matches first consumers: xq+wq (q-proj), xk+wv
        # (v-proj chunk 0), then wk (k-proj pair 0).
        nc.vector.memset(ones64, 1.0)
        qp0 = qproj(0)
        qp0[0]()                      # xq chunk 0 DMA first
        wdma(wq_sb, wqT, ND)()
        vchunks = [vproj_chunk(c) for c in range(NTC)]
        vchunks[0][0]()               # xk chunk 0 DMA
        wdma(wv_sb, wvT, ND)()
        wdma(wk_sb, wkT, ND)()
        for a in qp0[1]:
            a()
        for a in vchunks[0][1]:
            a()
        kp0 = [kproj_chunk(0, c) for c in range(NTC)]
        run(*kp0[0])

        # deadline-driven schedule for pair-0/qc0: chunk c of k-proj(p0)
        # and v-proj must be emitted by step 4c (their consumers); DMAs
        # go 8 steps early, compute spread over the 4 preceding steps.
        p0sched = [[] for _ in range(NTT)]
        for c in range(1, NTC):
            for pre, comp in (kp0[c], vchunks[c]):
                p0sched[max(0, 4 * c - 8)].append(pre)
                base = 4 * (c - 1)
                n = len(comp)
                for si in range(4):
                    lo, hi = (n * si) // 4, (n * (si + 1)) // 4
                    p0sched[base + si].extend(comp[lo:hi])
        qp1 = qproj(1)
        p0sched[0].append(qp1[0])
        for i, a in enumerate(qp1[1]):
            p0sched[1 + (i * 27) // len(qp1[1])].append(a)
        p0sched[8].append(wdma(wo_sb, woT, NM))

        # per-(pair, qc) filler schedules
        sched = {(0, 0): p0sched}
        sched[(0, 1)] = spread([kproj_chunk(1, c) for c in range(NTC)], NTT)
        s64 = spread([kproj_chunk(2, c) for c in range(NTC)], 2 * NTT)
        sched[(1, 0)], sched[(1, 1)] = s64[:NTT], s64[NTT:]
        s64 = spread([kproj_chunk(3, c) for c in range(NTC)], 2 * NTT)
        sched[(2, 0)], sched[(2, 1)] = s64[:NTT], s64[NTT:]
        sched[(3, 0)] = [[] for _ in range(NTT)]
        sched[(3, 1)] = spread([oproj_group(m8, 0) for m8 in range(D // P)],
                               NTT)

        # ================= attention (software-pipelined) ========
        for p in range(NM):
            for qc in range(NQC):
                qs = slice(qc * 512, (qc + 1) * 512)
                pvt = psp.tile([P, 512], F32, name="pvt", tag="pv", bufs=1)
                smt = psp.tile([P, 512], F32, name="smt", tag="sum", bufs=1)
                prev_ex = None

                def pv_sums(ex, t, p=p, pvt=pvt, smt=smt):
                    nc.tensor.matmul(
                        pvt[0:DH, :],
                        lhsT=v_sb[:, t, p * P:p * P + DH],
                        rhs=ex[:, 0, :],
                        start=(t == 0),
                        stop=(t == NTT - 1),
                        tile_position=(0, 0),
                    )
                    nc.tensor.matmul(
                        pvt[DH:P, :],
                        lhsT=v_sb[:, t, p * P + DH:(p + 1) * P],
                        rhs=ex[:, 1, :],
                        start=(t == 0),
                        stop=(t == NTT - 1),
                        tile_position=(0, 64),
                        skip_group_check=True,
                    )
                    nc.tensor.matmul(
                        smt[0:DH, :],
                        lhsT=ones64,
                        rhs=ex[:, 0, :],
                        start=(t == 0),
                        stop=(t == NTT - 1),
                        tile_position=(0, 0),
                    )
                    nc.tensor.matmul(
                        smt[DH:P, :],
                        lhsT=ones64,
                        rhs=ex[:, 1, :],
                        start=(t == 0),
                        stop=(t == NTT - 1),
                        tile_position=(0, 64),
                        skip_group_check=True,
                    )

                loop_sched = sched[(p, qc)]
                for t in range(NTT):
                    ts = slice(t * P, (t + 1) * P)
                    s_ps = psp.tile([P, 2, 512], F32, name="s_ps", tag="ps",
                                    bufs=2)
                    for hb in range(2):
                        base = 64 * hb
                        nc.tensor.matmul(
                            s_ps[:, hb, :],
                            lhsT=kT_sb[base:base + DH, p, ts],
                            rhs=qT_sb[base:base + DH, p, qs],
                            start=True,
                            stop=True,
                            tile_position=(base, 0),
                        )
                    ex = exp_pool.tile([P, 2, 512], BF16, name="ex", tag="ex")
                    nc.scalar.activation(
                        out=ex,
                        in_=s_ps,
                        func=mybir.ActivationFunctionType.Exp,
                        scale=SCALE,
                    )
                    # PE filler while ScalarE crunches exp
                    for a in loop_sched[t]:
                        a()
                    # software-pipelined PV+sums for the previous t
                    if prev_ex is not None:
                        pv_sums(prev_ex, t - 1)
                    prev_ex = ex
                pv_sums(prev_ex, NTT - 1)
                # normalize: attnT = outT * (1/sums)
                rec = small.tile([P, 512], F32, name="rec", tag="rec")
                nc.vector.reciprocal_approx_fast(out=rec, in_=smt)
                nc.vector.tensor_mul(at_sb[:, p, qs], pvt[:, :], rec)

        # ================= coda: o-projection for qc1 =================
        for m8 in range(D // P):
            run(*oproj_group(m8, 1))


_CACHED_NC = None


def build_program():
    global _CACHED_NC
    if _CACHED_NC is not None:
        return _CACHED_NC
    nc = bacc.Bacc(
        "TRN2", target_bir_lowering=False, debug=False, num_devices=NCORES
    )
    xqT = nc.dram_tensor("xqT", [D, Q], BF16, kind="ExternalInput").ap()
    xkT = nc.dram_tensor("xkT", [D, T], BF16, kind="ExternalInput").ap()
    wqT = nc.dram_tensor("wqT", [D, F], BF16, kind="ExternalInput").ap()
    wkT = nc.dram_tensor("wkT", [D, F], BF16, kind="ExternalInput").ap()
    wvT = nc.dram_tensor("wvT", [D, F], BF16, kind="ExternalInput").ap()
    woT = nc.dram_tensor("woT", [F, D], BF16, kind="ExternalInput").ap()
    yT = nc.dram_tensor("yT", [D, Q], F32, kind="ExternalOutput").ap()
    with tile.TileContext(nc) as tc:
        _emit_kernel(nc, tc, xqT, xkT, wqT, wkT, wvT, woT, yT)
    nc.compile()
    _CACHED_NC = nc
    return nc


def make_in_maps(q_in, kv_in, Wq, Wk, Wv, Wo):
    """Shard + transpose + cast on host. Core = b*2 + g."""
    in_maps = []
    xqTs, xkTs = [], []
    for b in range(B):
        xqTs.append(np.ascontiguousarray(q_in[b].T).astype(NPBF16))
        xkTs.append(np.ascontiguousarray(kv_in[b].T).astype(NPBF16))
    w_parts = []
    for g in range(G):
        blk = slice(g * F, (g + 1) * F)
        w_parts.append(
            dict(
                wqT=np.ascontiguousarray(Wq[blk, :].T).astype(NPBF16),
                wkT=np.ascontiguousarray(Wk[blk, :].T).astype(NPBF16),
                wvT=np.ascontiguousarray(Wv[blk, :].T).astype(NPBF16),
                woT=np.ascontiguousarray(Wo[:, blk].T).astype(NPBF16),
            )
        )
    for b in range(B):
        for g in range(G):
            m = dict(xqT=xqTs[b], xkT=xkTs[b])
            m.update(w_parts[g])
            in_maps.append(m)
    return in_maps


def assemble_output(results):
    """results: list of per-core dicts with 'yT' [D, Q] fp32 partials."""
    out = np.empty((B, Q, D), dtype=np.float32)
    for b in range(B):
        acc = results[2 * b]["yT"] + results[2 * b + 1]["yT"]
        out[b] = acc.T
    return out


def kernel(q_in, kv_in, Wq, Wk, Wv, Wo):
    q_in = np.asarray(q_in, dtype=np.float32)
    kv_in = np.asarray(kv_in, dtype=np.float32)
    Wq = np.asarray(Wq, dtype=np.float32)
    Wk = np.asarray(Wk, dtype=np.float32)
    Wv = np.asarray(Wv, dtype=np.float32)
    Wo = np.asarray(Wo, dtype=np.float32)
    nc = build_program()
    in_maps = make_in_maps(q_in, kv_in, Wq, Wk, Wv, Wo)
    res = run_bass_kernel_spmd(nc, in_maps, list(range(NCORES)))
    return assemble_output(res.results)


# revision 18
# speedup vs baseline: 1.0690x; 1.0370x over previous
"""Trainium2 Bass kernel for nn_CrossAttention (B=4, Q=1024, T=4096, D=1024, H=16).

Sharding: core = b*2 + g  (b in 0..3 batches, g in 0..1 head-groups of 8 heads).
Each core computes, for its (batch, head-group):
  qT = (Wq_g @ x_q.T)          [512, Q]   (feature-major; head pairs stacked)
  kT = (Wk_g @ x_kv.T)         [512, T]
  v  = (x_kv @ Wv_g.T)         [T, 512]
  sT = k_h @ q_h.T             [T, Q] per head  (scores transposed)
  p  = exp(sT / 8)             (softmax w/o max-subtraction; scores ~N(0,1))
  outT_h = v_h.T @ p ; sums_h = ones.T @ p ; attnT_h = outT_h * (1/sums_h)
  yT_partial = Wo[:, gblock].T.T @ attnT  -> [1024, Q]  fp32
Host sums the two head-group partials per batch and transposes.

The emission is software-pipelined: the attention loop over kv tiles is
ScalarE(exp)-bound, so all projection work (k-proj of the next head pair,
v/q projections, o-proj) is sliced into small actions and emitted inside
the attention loop as TensorE filler, paced so producers stay ahead of
their consumers.
"""

import sys

import numpy as np

for _p in ("/opt/trn_rl_repo",):
    if _p not in sys.path:
        sys.path.insert(0, _p)

import ml_dtypes

import concourse.bass as bass
import concourse.tile as tile
from concourse import bacc, mybir
from concourse.bass_utils import run_bass_kernel_spmd

BF16 = mybir.dt.bfloat16
F32 = mybir.dt.float32
NPBF16 = np.dtype(ml_dtypes.bfloat16)

D = 1024          # model dim
Q = 1024          # query length
T = 4096          # kv length
B = 4             # batch
H = 16            # heads
DH = 64           # head dim
NCORES = 8
G = 2             # head groups (cores per batch)
F = D // G        # features per core = 512
P = 128
ND = D // P       # 8 d-tiles (contraction tiles for projections)
NM = F // P       # 4 feature tiles (head pairs)
NQC = Q // 512    # 2 query chunks
NTC = T // 512    # 8 kv chunks
NTT = T // P      # 32 kv tiles
SCALE = DH ** -0.5


def _emit_kernel(nc, tc, xqT, xkT, wqT, wkT, wvT, woT, yT):
    from contextlib import ExitStack

    ctx = ExitStack()
    with ctx:
        wp = ctx.enter_context(tc.tile_pool(name="wp", bufs=1))
        xp = ctx.enter_context(tc.tile_pool(name="xp", bufs=2))
        st = ctx.enter_context(tc.tile_pool(name="st", bufs=1))
        exp_pool = ctx.enter_context(tc.tile_pool(name="exp", bufs=4))
        small = ctx.enter_context(tc.tile_pool(name="small", bufs=2))
        yop = ctx.enter_context(tc.tile_pool(name="yop", bufs=4))
        psp = ctx.enter_context(tc.tile_pool(name="psp", bufs=1, space="PSUM"))

        # ---- resident weights / activations ----
        wq_sb = wp.tile([P, ND, F], BF16, name="wq_sb", tag="wq")
        wk_sb = wp.tile([P, ND, F], BF16, name="wk_sb", tag="wk")
        wv_sb = wp.tile([P, ND, F], BF16, name="wv_sb", tag="wv")
        wo_sb = wp.tile([P, NM, D], BF16, name="wo_sb", tag="wo")
        qT_sb = st.tile([P, NM, Q], BF16, name="qT_sb", tag="qT")
        kT_sb = st.tile([P, NM, T], BF16, name="kT_sb", tag="kT")
        v_sb = st.tile([P, NTT, F], BF16, name="v_sb", tag="v")
        at_sb = st.tile([P, NM, Q], BF16, name="at_sb", tag="at")
        ones64 = st.tile([P, DH], BF16, name="ones64", tag="ones")

        def wdma(w_sb, wT, n):
            def act():
                for d in range(n):
                    nc.sync.dma_start(
                        out=w_sb[:, d, :], in_=wT[d * P:(d + 1) * P, :]
                    )
            return act

        # ---- projection emitters: (pre_action, [compute actions]) ----
        def kproj_chunk(p, tc_i):
            state = {}

            def dma():
                xk2 = xp.tile([P, ND, 512], BF16, name="xk2", tag="xk2")
                for d in range(ND):
                    nc.sync.dma_start(
                        out=xk2[:, d, :],
                        in_=xkT[d * P:(d + 1) * P,
                                tc_i * 512:(tc_i + 1) * 512],
                    )
                state["xk2"] = xk2

            comp = []

            def alloc():
                state["pk"] = psp.tile([P, 512], F32, name="pk", tag="pp",
                                       bufs=2)

            comp.append(alloc)
            for d in range(ND):
                def mm(d=d):
                    nc.tensor.matmul(
                        state["pk"],
                        lhsT=wk_sb[:, d, p * P:(p + 1) * P],
                        rhs=state["xk2"][:, d, :],
                        start=(d == 0),
                        stop=(d == ND - 1),
                    )
                comp.append(mm)

            def cp():
                nc.vector.tensor_copy(
                    out=kT_sb[:, p, tc_i * 512:(tc_i + 1) * 512],
                    in_=state["pk"],
                )
            comp.append(cp)
            return dma, comp

        def vproj_chunk(tc_i):
            state = {}

            def dma():
                xk_t = xp.tile([P, ND, 512], BF16, name="xk_t", tag="xk")
                for d in range(ND):
                    nc.sync.dma_start(
                        out=xk_t[:, d, :],
                        in_=xkT[d * P:(d + 1) * P,
                                tc_i * 512:(tc_i + 1) * 512],
                    )
                state["xk"] = xk_t

            comp = []
            for j in range(4):
                def alloc(j=j):
                    state[j] = psp.tile([P, 512], F32, name="pv", tag="pp",
                                        bufs=2)
                comp.append(alloc)
                for d in range(ND):
                    def mm(j=j, d=d):
                        nc.tensor.matmul(
                            state[j],
                            lhsT=state["xk"][:, d, j * P:(j + 1) * P],
                            rhs=wv_sb[:, d, :],
                            start=(d == 0),
                            stop=(d == ND - 1),
                        )
                    comp.append(mm)

                def cp(j=j):
                    nc.vector.tensor_copy(
                        out=v_sb[:, tc_i * 4 + j, :], in_=state[j]
                    )
                comp.append(cp)
            return dma, comp

        def qproj(qc):
            state = {}

            def dma():
                xq_t = xp.tile([P, ND, 512], BF16, name="xq_t", tag="xq")
                for d in range(ND):
                    nc.sync.dma_start(
                        out=xq_t[:, d, :],
                        in_=xqT[d * P:(d + 1) * P, qc * 512:(qc + 1) * 512],
                    )
                state["xq"] = xq_t

            comp = []
            for m in range(NM):
                def alloc(m=m):
                    state[m] = psp.tile([P, 512], F32, name="pq", tag="pp",
                                        bufs=2)
                comp.append(alloc)
                for d in range(ND):
                    def mm(m=m, d=d):
                        nc.tensor.matmul(
                            state[m],
                            lhsT=wq_sb[:, d, m * P:(m + 1) * P],
                            rhs=state["xq"][:, d, :],
                            start=(d == 0),
                            stop=(d == ND - 1),
                        )
                    comp.append(mm)

                def cp(m=m):
                    nc.vector.tensor_copy(
                        out=qT_sb[:, m, qc * 512:(qc + 1) * 512],
                        in_=state[m],
                    )
                comp.append(cp)
            return dma, comp

        def oproj_group(m8, qc):
            state = {}
            comp = []

            def alloc():
                state["py"] = psp.tile([P, 512], F32, name="py", tag="pp",
                                       bufs=2)
            comp.append(alloc)
            for k in range(NM):
                def mm(k=k):
                    nc.tensor.matmul(
                        state["py"],
                        lhsT=wo_sb[:, k, m8 * P:(m8 + 1) * P],
                        rhs=at_sb[:, k, qc * 512:(qc + 1) * 512],
                        start=(k == 0),
                        stop=(k == NM - 1),
                    )
                comp.append(mm)

            def st_dma():
                y_t = yop.tile([P, 512], F32, name="y_t", tag="y")
                nc.vector.tensor_copy(out=y_t, in_=state["py"])
                nc.sync.dma_start(
                    out=yT[m8 * P:(m8 + 1) * P, qc * 512:(qc + 1) * 512],
                    in_=y_t,
                )
            comp.append(st_dma)
            return None, comp

        def run(pre, comp):
            if pre is not None:
                pre()
            for a in comp:
                a()

        def spread(pairs, nsteps, lead=4):
            """Evenly distribute (pre, comp) groups over nsteps slots;
            pre (DMA) actions are placed `lead` slots before the group's
            first compute action."""
            sched = [[] for _ in range(nsteps)]
            total = sum(len(c) for _, c in pairs) or 1
            pos = 0
            for pre, comp in pairs:
                first = (pos * nsteps) // total
                if pre is not None:
                    sched[max(0, first - lead)].append(pre)
                for a in comp:
                    sched[min(nsteps - 1, (pos * nsteps) // total)].append(a)
                    pos += 1
            return sched

        # ================= prologue =================
        # DMA order matches first consumers: xq+wq (q-proj), xk+wv
        # (v-proj chunk 0), then wk (k-proj pair 0).
        nc.vector.memset(ones64, 1.0)
        qp0 = qproj(0)
        qp0[0]()                      # xq chunk 0 DMA first
        wdma(wq_sb, wqT, ND)()
        vchunks = [vproj_chunk(c) for c in range(NTC)]
        vchunks[0][0]()               # xk chunk 0 DMA
        wdma(wv_sb, wvT, ND)()
        wdma(wk_sb, wkT, ND)()
        for a in qp0[1]:
            a()
        for a in vchunks[0][1]:
            a()
        kp0 = [kproj_chunk(0, c) for c in range(NTC)]
        run(*kp0[0])

        # deadline-driven schedule for pair-0/qc0: chunk c of k-proj(p0)
        # and v-proj must be emitted by step 4c (their consumers); DMAs
        # go 8 steps early, compute spread over the 4 preceding steps.
        p0sched = [[] for _ in range(NTT)]
        for c in range(1, NTC):
            for pre, comp in (kp0[c], vchunks[c]):
                p0sched[max(0, 4 * c - 8)].append(pre)
                base = 4 * (c - 1)
                n = len(comp)
                for si in range(4):
                    lo, hi = (n * si) // 4, (n * (si + 1)) // 4
                    p0sched[base + si].extend(comp[lo:hi])
        qp1 = qproj(1)
        p0sched[0].append(qp1[0])
        for i, a in enumerate(qp1[1]):
            p0sched[1 + (i * 27) // len(qp1[1])].append(a)
        p0sched[8].append(wdma(wo_sb, woT, NM))

        # per-(pair, qc) filler schedules
        sched = {(0, 0): p0sched}
        sched[(0, 1)] = spread([kproj_chunk(1, c) for c in range(NTC)], NTT)
        s64 = spread([kproj_chunk(2, c) for c in range(NTC)], 2 * NTT)
        sched[(1, 0)], sched[(1, 1)] = s64[:NTT], s64[NTT:]
        s64 = spread([kproj_chunk(3, c) for c in range(NTC)], 2 * NTT)
        sched[(2, 0)], sched[(2, 1)] = s64[:NTT], s64[NTT:]
        sched[(3, 0)] = [[] for _ in range(NTT)]
        sched[(3, 1)] = spread([oproj_group(m8, 0) for m8 in range(D // P)],
                               NTT)

        # ================= attention (software-pipelined) ========
        for p in range(NM):
            for qc in range(NQC):
                qs = slice(qc * 512, (qc + 1) * 512)
                pvt = psp.tile([P, 512], F32, name="pvt", tag="pv", bufs=1)
                smt = psp.tile([P, 512], F32, name="smt", tag="sum", bufs=1)
                prev_ex = None

                def pv_sums(ex, t, p=p, pvt=pvt, smt=smt):
                    nc.tensor.matmul(
                        pvt[0:DH, :],
                        lhsT=v_sb[:, t, p * P:p * P + DH],
                        rhs=ex[:, 0, :],
                        start=(t == 0),
                        stop=(t == NTT - 1),
                        tile_position=(0, 0),
                    )
                    nc.tensor.matmul(
                        pvt[DH:P, :],
                        lhsT=v_sb[:, t, p * P + DH:(p + 1) * P],
                        rhs=ex[:, 1, :],
                        start=(t == 0),
                        stop=(t == NTT - 1),
                        tile_position=(0, 64),
                        skip_group_check=True,
                    )
                    nc.tensor.matmul(
                        smt[0:DH, :],
                        lhsT=ones64,
                        rhs=ex[:, 0, :],
                        start=(t == 0),
                        stop=(t == NTT - 1),
                        tile_position=(0, 0),
                    )
                    nc.tensor.matmul(
                        smt[DH:P, :],
                        lhsT=ones64,
                        rhs=ex[:, 1, :],
                        start=(t == 0),
                        stop=(t == NTT - 1),
                        tile_position=(0, 64),
                        skip_group_check=True,
                    )

                def qk_exp(t):
                    ts = slice(t * P, (t + 1) * P)
                    s_ps = psp.tile([P, 2, 512], F32, name="s_ps", tag="ps",
                                    bufs=2)
                    for hb in range(2):
                        base = 64 * hb
                        nc.tensor.matmul(
                            s_ps[:, hb, :],
                            lhsT=kT_sb[base:base + DH, p, ts],
                            rhs=qT_sb[base:base + DH, p, qs],
                            start=True,
                            stop=True,
                            tile_position=(base, 0),
                        )
                    ex = exp_pool.tile([P, 2, 512], BF16, name="ex", tag="ex")
                    nc.scalar.activation(
                        out=ex,
                        in_=s_ps,
                        func=mybir.ActivationFunctionType.Exp,
                        scale=SCALE,
                    )
                    return ex

                loop_sched = sched[(p, qc)]
                prev = []
                for ti in range(0, NTT, 2):
                    exs = [qk_exp(ti), qk_exp(ti + 1)]
                    # PE filler while ScalarE crunches exp
                    for a in loop_sched[ti] + loop_sched[ti + 1]:
                        a()
                    # software-pipelined PV+sums for the previous t pair
                    for pe, pt in prev:
                        pv_sums(pe, pt)
                    prev = [(exs[0], ti), (exs[1], ti + 1)]
                for pe, pt in prev:
                    pv_sums(pe, pt)
                # normalize: attnT = outT * (1/sums)
                rec = small.tile([P, 512], F32, name="rec", tag="rec")
                nc.vector.reciprocal_approx_fast(out=rec, in_=smt)
                nc.vector.tensor_mul(at_sb[:, p, qs], pvt[:, :], rec)

        # ================= coda: o-projection for qc1 =================
        for m8 in range(D // P):
            run(*oproj_group(m8, 1))


_CACHED_NC = None


def build_program():
    global _CACHED_NC
    if _CACHED_NC is not None:
        return _CACHED_NC
    nc = bacc.Bacc(
        "TRN2", target_bir_lowering=False, debug=False, num_devices=NCORES
    )
    xqT = nc.dram_tensor("xqT", [D, Q], BF16, kind="ExternalInput").ap()
    xkT = nc.dram_tensor("xkT", [D, T], BF16, kind="ExternalInput").ap()
    wqT = nc.dram_tensor("wqT", [D, F], BF16, kind="ExternalInput").ap()
    wkT = nc.dram_tensor("wkT", [D, F], BF16, kind="ExternalInput").ap()
    wvT = nc.dram_tensor("wvT", [D, F], BF16, kind="ExternalInput").ap()
    woT = nc.dram_tensor("woT", [F, D], BF16, kind="ExternalInput").ap()
    yT = nc.dram_tensor("yT", [D, Q], F32, kind="ExternalOutput").ap()
    with tile.TileContext(nc) as tc:
        _emit_kernel(nc, tc, xqT, xkT, wqT, wkT, wvT, woT, yT)
    nc.compile()
    _CACHED_NC = nc
    return nc


def make_in_maps(q_in, kv_in, Wq, Wk, Wv, Wo):
    """Shard + transpose + cast on host. Core = b*2 + g."""
    in_maps = []
    xqTs, xkTs = [], []
    for b in range(B):
        xqTs.append(np.ascontiguousarray(q_in[b].T).astype(NPBF16))
        xkTs.append(np.ascontiguousarray(kv_in[b].T).astype(NPBF16))
    w_parts = []
    for g in range(G):
        blk = slice(g * F, (g + 1) * F)
        w_parts.append(
            dict(
                wqT=np.ascontiguousarray(Wq[blk, :].T).astype(NPBF16),
                wkT=np.ascontiguousarray(Wk[blk, :].T).astype(NPBF16),
                wvT=np.ascontiguousarray(Wv[blk, :].T).astype(NPBF16),
                woT=np.ascontiguousarray(Wo[:, blk].T).astype(NPBF16),
            )
        )
    for b in range(B):
        for g in range(G):
            m = dict(xqT=xqTs[b], xkT=xkTs[b])
            m.update(w_parts[g])
            in_maps.append(m)
    return in_maps


def assemble_output(results):
    """results: list of per-core dicts with 'yT' [D, Q] fp32 partials."""
    out = np.empty((B, Q, D), dtype=np.float32)
    for b in range(B):
        acc = results[2 * b]["yT"] + results[2 * b + 1]["yT"]
        out[b] = acc.T
    return out


def kernel(q_in, kv_in, Wq, Wk, Wv, Wo):
    q_in = np.asarray(q_in, dtype=np.float32)
    kv_in = np.asarray(kv_in, dtype=np.float32)
    Wq = np.asarray(Wq, dtype=np.float32)
    Wk = np.asarray(Wk, dtype=np.float32)
    Wv = np.asarray(Wv, dtype=np.float32)
    Wo = np.asarray(Wo, dtype=np.float32)
    nc = build_program()
    in_maps = make_in_maps(q_in, kv_in, Wq, Wk, Wv, Wo)
    res = run_bass_kernel_spmd(nc, in_maps, list(range(NCORES)))
    return assemble_output(res.results)
